# revision 3
# baseline (speedup 1.0000x reference)
"""Trainium2 Bass kernel v2 for gnn_message_passing nn_CNNTest_10299331576114.

V=100000 vertices sharded over 8 cores (12500 each), two NEFFs.

Stage 1: gather g = vp[nb1] (batched indirect DMA), conv-k3 over the
neighbor axis as a banded matmul ([33,32] incl bias row), relu, sum ->
h shard (mean 1/32 folded downstream).

Stage 2 (v2 redesign vs v1):
 - single bf16 h2-strip [32, EXT]; the 3-shift vertex conv is done by 3
   accumulating PSUM matmuls over shifted strip windows (no 3x DVE copies)
 - bias rows come from pool buffers whose last row is prefilled with 1.0
   once outside the repeat loop (no per-tile memsets)
 - mbig/wfcb/w2k3 matmuls in bf16 (2x PE throughput; tol is 2e-2)
 - PSUM split into per-stage pools (psc/pst1/pst2/psf/psl) sized to the
   8-bank budget for deeper cross-tile pipelining
 - PSUM evacuation copies on ACT (scalar) engine, reduce/scale on DVE
"""

import time

import numpy as np

import concourse.bacc as bacc
import concourse.mybir as mybir
import concourse.tile as tile
from concourse import bass
from concourse.bass import IndirectOffsetOnAxis
from concourse.bass_utils import run_bass_kernel_spmd
from concourse.masks import make_identity

F32 = mybir.dt.float32
BF16 = mybir.dt.bfloat16
I32 = mybir.dt.int32
AX = mybir.AxisListType
ALU = mybir.AluOpType
ACTF = mybir.ActivationFunctionType
NPBF16 = mybir.dt.np(BF16)

V = 100000
N = 32
NCORES = 8
VC = V // NCORES          # 12500
P = 125                   # vertices per tile
T1 = VC // P              # 100 tiles per core
EXT = VC + 2              # stage-2 extended range (one halo vertex per side)

_CACHE = {}
TIMES = {}
_LAST_INPUTS = None

S1_GB = 4                 # stage-1 gather batch (tiles per indirect DMA)
S2_GB = 4                 # stage-2 gather batch


def _build_stage1(repeat=1, gb=4, psg_b=2, psc_b=2):
    nc = bacc.Bacc("TRN2", target_bir_lowering=False, debug=False,
                   num_devices=NCORES)
    vp = nc.dram_tensor("vp", [V], F32, kind="ExternalInput")
    nb1 = nc.dram_tensor("nb1", [VC, N], I32, kind="ExternalInput")
    a1 = nc.dram_tensor("a1", [N + 1, N], F32, kind="ExternalInput")
    hsh = nc.dram_tensor("hsh", [VC], F32, kind="ExternalOutput")

    with tile.TileContext(nc) as tc:
        with (
            tc.tile_pool(name="const", bufs=1) as cp,
            tc.tile_pool(name="io", bufs=4) as iop,
            tc.tile_pool(name="work", bufs=4) as wp,
            tc.tile_pool(name="hc", bufs=1) as hcp,
            tc.tile_pool(name="psg", bufs=psg_b, space="PSUM") as psg,
            tc.tile_pool(name="psc", bufs=psc_b, space="PSUM") as psc,
            tc.tile_pool(name="psb", bufs=1, space="PSUM") as psb,
        ):
            ident = cp.tile([128, 128], F32)
            make_identity(nc, ident[:])
            a1t = cp.tile([N + 1, N], F32)
            nc.sync.dma_start(a1t[:], a1[:])
            hcol = hcp.tile([P, T1], F32)

            rep = tc.For_i(0, repeat, 1) if repeat > 1 else None
            if rep is not None:
                rep.__enter__()
            t = 0
            for nb_batch in ([gb] * (T1 // gb) + ([T1 % gb] if T1 % gb else [])):
                it = iop.tile([P, N * gb], I32, tag="idx")
                nc.sync.dma_start(
                    it[:, :N * nb_batch].rearrange("p (b n) -> p b n", n=N),
                    nb1[P * t:P * (t + nb_batch), :].rearrange(
                        "(b p) n -> p b n", p=P))
                g = wp.tile([P, N * gb], F32, tag="g")
                nc.gpsimd.indirect_dma_start(
                    out=g[:, :N * nb_batch], out_offset=None, in_=vp[:, None],
                    in_offset=IndirectOffsetOnAxis(ap=it[:, :N * nb_batch],
                                                   axis=0))
                for b in range(nb_batch):
                    gtp = psg.tile([N, P], F32, tag="gt")
                    nc.tensor.transpose(gtp[:], g[:, N * b:N * (b + 1)],
                                        ident[:P, :P])
                    gt = wp.tile([N + 1, P], F32, tag="gts")
                    nc.vector.tensor_copy(gt[:N, :], gtp[:])
                    nc.vector.memset(gt[N:N + 1, :], 1.0)
                    c1p = psc.tile([P, N], F32, tag="c1")
                    nc.tensor.matmul(c1p[:], lhsT=gt[:], rhs=a1t[:],
                                     start=True, stop=True)
                    r = wp.tile([P, N], F32, tag="r")
                    nc.scalar.activation(r[:], c1p[:], ACTF.Relu)
                    nc.vector.reduce_sum(hcol[:, t:t + 1], r[:], axis=AX.X)
                    t += 1

            if rep is not None:
                rep.__exit__(None, None, None)
            htp = psb.tile([T1, P], F32)
            nc.tensor.transpose(htp[:], hcol[:], ident[:P, :P])
            hst = wp.tile([T1, P], F32, tag="hst")
            nc.vector.tensor_copy(hst[:], htp[:])
            nc.sync.dma_start(
                hsh[:].rearrange("(t p) -> t p", p=P), hst[:])
    nc.finalize()
    return nc


def _build_stage2(repeat=1, bench_internal_out=False, gb=S2_GB,
                  psum_cfg=None, relu_split=True, tts_dve=False,
                  sbuf_b=4, out_batch=False):
    cfg = dict(psc=2, pst1=1, pst2=2, psf=1, psl=2)
    if psum_cfg:
        cfg.update(psum_cfg)
    nc = bacc.Bacc("TRN2", target_bir_lowering=False, debug=False,
                   num_devices=NCORES)
    hp = nc.dram_tensor("hp", [V + 2], F32, kind="ExternalInput")
    nb2e = nc.dram_tensor("nb2e", [EXT, N], I32, kind="ExternalInput")
    mbig = nc.dram_tensor("mbig", [97, 1024], BF16, kind="ExternalInput")
    w2k3 = nc.dram_tensor("w2k3", [32, 192], BF16, kind="ExternalInput")
    wfcb = nc.dram_tensor("wfcb", [65, 512], BF16, kind="ExternalInput")
    mask2 = nc.dram_tensor("mask2", [32, 2], BF16, kind="ExternalInput")
    if bench_internal_out:
        out = nc.dram_tensor("out", [VC, 512], F32)
        tiny = nc.dram_tensor("tiny", [1, 1], F32, kind="ExternalOutput")
    else:
        out = nc.dram_tensor("out", [VC, 512], F32, kind="ExternalOutput")
        tiny = None

    NB = T1 // gb             # full batches
    assert T1 % gb == 0

    with tile.TileContext(nc) as tc:
        with (
            tc.tile_pool(name="const", bufs=1) as cp,
            tc.tile_pool(name="strip", bufs=1) as sp,
            tc.tile_pool(name="io", bufs=sbuf_b) as iop,
            tc.tile_pool(name="work", bufs=sbuf_b) as wp,
            tc.tile_pool(name="tts", bufs=sbuf_b) as ttsp,
            tc.tile_pool(name="f2s", bufs=2) as f2sp,
            tc.tile_pool(name="big", bufs=sbuf_b) as bp,
            tc.tile_pool(name="psc", bufs=cfg["psc"], space="PSUM") as psc,
            tc.tile_pool(name="pst1", bufs=cfg["pst1"], space="PSUM") as pst1,
            tc.tile_pool(name="pst2", bufs=cfg["pst2"], space="PSUM") as pst2,
            tc.tile_pool(name="psf", bufs=cfg["psf"], space="PSUM") as psf,
            tc.tile_pool(name="psl", bufs=cfg["psl"], space="PSUM") as psl,
        ):
            ident = cp.tile([128, 128], F32)
            make_identity(nc, ident[:])
            mbigt = cp.tile([97, 1024], BF16)
            nc.sync.dma_start(mbigt[:], mbig[:])
            w2all = cp.tile([32, 192], BF16)
            nc.sync.dma_start(w2all[:], w2k3[:])
            wfcbt = cp.tile([65, 512], BF16)
            nc.sync.dma_start(wfcbt[:], wfcb[:])
            m2t = cp.tile([32, 2], BF16)
            nc.sync.dma_start(m2t[:], mask2[:])

            # single shifted strip: col s = h2 of extended vertex s (bf16)
            h2strip = sp.tile([32, EXT], BF16)

            # prefill ones rows (persist across loop iterations)
            for _ in range(sbuf_b):
                ttspre = ttsp.tile([97, P], BF16, tag="tts")
                nc.vector.memset(ttspre[96:97, :], 1.0)
            for _ in range(2):
                f2spre = f2sp.tile([65, 500], BF16, tag="f2s")
                nc.vector.memset(f2spre[64:65, :], 1.0)

            rep = tc.For_i(0, repeat, 1) if repeat > 1 else None
            if rep is not None:
                rep.__enter__()

            def compute_tile(tt_ap, t):
                # tt_ap: [P, 96] gathered triples for extended tile t
                ttp = pst1.tile([96, P], F32, tag="tp1")
                nc.tensor.transpose(ttp[:], tt_ap, ident[:P, :P])
                tts = ttsp.tile([97, P], BF16, tag="tts")
                if tts_dve:
                    nc.vector.tensor_copy(tts[:96, :], ttp[:])
                else:
                    nc.scalar.activation(tts[:96, :], ttp[:], ACTF.Copy)
                cr = bp.tile([P, 1024], BF16, tag="cr")
                for h in range(2):
                    cps = psc.tile([P, 512], F32, tag="c")
                    nc.tensor.matmul(cps[:], lhsT=tts[:],
                                     rhs=mbigt[:, 512 * h:512 * (h + 1)],
                                     start=True, stop=True)
                    if relu_split and h == 1:
                        nc.vector.tensor_scalar(
                            out=cr[:, 512 * h:512 * (h + 1)], in0=cps[:],
                            scalar1=0.0, scalar2=None, op0=ALU.max)
                    else:
                        nc.scalar.activation(cr[:, 512 * h:512 * (h + 1)],
                                             cps[:], ACTF.Relu)
                h2 = wp.tile([P, N], F32, tag="h2w")
                nc.vector.reduce_sum(
                    h2[:], cr[:].rearrange("p (c j) -> p c j", j=32),
                    axis=AX.X)
                h2p = pst2.tile([N, P], F32, tag="tp2")
                nc.tensor.transpose(h2p[:], h2[:], ident[:P, :P])
                nc.vector.tensor_copy(h2strip[:, P * t:P * (t + 1)], h2p[:])

            def phase_a_batch(t0):
                it = iop.tile([P, N * gb], I32, tag="idx")
                nc.sync.dma_start(
                    it[:].rearrange("p (b n) -> p b n", n=N),
                    nb2e[P * t0:P * (t0 + gb), :].rearrange(
                        "(b p) n -> p b n", p=P))
                tt = wp.tile([P, 3 * N * gb], F32, tag="tt")
                nc.gpsimd.indirect_dma_start(
                    out=tt[:], out_offset=None, in_=hp[:, None],
                    in_offset=IndirectOffsetOnAxis(ap=it[:], axis=0))
                for b in range(gb):
                    compute_tile(tt[:, 96 * b:96 * (b + 1)], t0 + b)

            def phase_a_last():
                # last 125 extended rows (overlaps tail of regular tiles)
                ot = EXT - P
                it = iop.tile([P, N], I32, tag="idxl")
                nc.sync.dma_start(it[:], nb2e[ot:ot + P, :])
                tt = wp.tile([P, 3 * N], F32, tag="ttl")
                nc.gpsimd.indirect_dma_start(
                    out=tt[:], out_offset=None, in_=hp[:, None],
                    in_offset=IndirectOffsetOnAxis(ap=it[:], axis=0))
                ttp = pst1.tile([96, P], F32, tag="tp1")
                nc.tensor.transpose(ttp[:], tt[:], ident[:P, :P])
                tts = ttsp.tile([97, P], BF16, tag="tts")
                if tts_dve:
                    nc.vector.tensor_copy(tts[:96, :], ttp[:])
                else:
                    nc.scalar.activation(tts[:96, :], ttp[:], ACTF.Copy)
                cr = bp.tile([P, 1024], BF16, tag="cr")
                for h in range(2):
                    cps = psc.tile([P, 512], F32, tag="c")
                    nc.tensor.matmul(cps[:], lhsT=tts[:],
                                     rhs=mbigt[:, 512 * h:512 * (h + 1)],
                                     start=True, stop=True)
                    if relu_split and h == 1:
                        nc.vector.tensor_scalar(
                            out=cr[:, 512 * h:512 * (h + 1)], in0=cps[:],
                            scalar1=0.0, scalar2=None, op0=ALU.max)
                    else:
                        nc.scalar.activation(cr[:, 512 * h:512 * (h + 1)],
                                             cps[:], ACTF.Relu)
                h2 = wp.tile([P, N], F32, tag="h2w")
                nc.vector.reduce_sum(
                    h2[:], cr[:].rearrange("p (c j) -> p c j", j=32),
                    axis=AX.X)
                h2p = pst2.tile([N, P], F32, tag="tp2")
                nc.tensor.transpose(h2p[:], h2[:], ident[:P, :P])
                nc.vector.tensor_copy(h2strip[:, ot:EXT], h2p[:])

            def phase_b_group(g):
                # logits for local vertices [500g, 500g+500):
                # f2[o, s] = sum_r sum_q w2k3[32r+q, o] * strip[q, 500g+s+r]
                f2p = psf.tile([64, 500], F32, tag="f2")
                for r in range(3):
                    nc.tensor.matmul(
                        f2p[:], lhsT=w2all[:, 64 * r:64 * (r + 1)],
                        rhs=h2strip[:, 500 * g + r:500 * (g + 1) + r],
                        start=(r == 0), stop=(r == 2))
                f2s = f2sp.tile([65, 500], BF16, tag="f2s")
                nc.scalar.activation(f2s[:64, :], f2p[:], ACTF.Copy)
                o4 = bp.tile([P, 2048], F32, tag="o") if out_batch else None
                for b in range(4):
                    t = 4 * g + b
                    lgp = psl.tile([P, 512], F32, tag="lg")
                    nc.tensor.matmul(lgp[:], lhsT=f2s[:, P * b:P * (b + 1)],
                                     rhs=wfcbt[:], start=True, stop=True)
                    e = bp.tile([P, 512], BF16, tag="e")
                    ssum = wp.tile([P, 1], F32, tag="ss")
                    nc.scalar.activation(e[:], lgp[:], ACTF.Exp,
                                         accum_out=ssum[:])
                    rinv = wp.tile([P, 1], F32, tag="ri")
                    nc.vector.reciprocal(rinv[:], ssum[:])
                    if out_batch:
                        nc.vector.tensor_scalar(
                            out=o4[:, 512 * b:512 * (b + 1)], in0=e[:],
                            scalar1=rinv[:], scalar2=None, op0=ALU.mult)
                    else:
                        o = bp.tile([P, 512], F32, tag="o")
                        nc.vector.tensor_scalar(out=o[:], in0=e[:],
                                                scalar1=rinv[:], scalar2=None,
                                                op0=ALU.mult)
                        nc.sync.dma_start(out[bass.ts(t, P), :], o[:])
                if out_batch:
                    nc.sync.dma_start(
                        out[500 * g:500 * (g + 1), :].rearrange(
                            "(b p) c -> p b c", p=P),
                        o4[:].rearrange("p (b c) -> p b c", c=512))

            done_b = 0
            for k in range(NB):
                phase_a_batch(gb * k)
                if k == 0:
                    # left halo: zero col 0 for core 0 (mask input)
                    nc.vector.tensor_tensor(
                        out=h2strip[:, 0:1], in0=h2strip[:, 0:1],
                        in1=m2t[:, 0:1], op=ALU.mult)
                # group g needs strip cols through 500(g+1)+1, i.e. tiles
                # through (500(g+1)+2)/125
                while 500 * (done_b + 1) + 2 <= P * gb * (k + 1):
                    phase_b_group(done_b)
                    done_b += 1
            phase_a_last()
            # right halo: zero col EXT-1 for core 7
            nc.vector.tensor_tensor(
                out=h2strip[:, EXT - 1:EXT], in0=h2strip[:, EXT - 1:EXT],
                in1=m2t[:, 1:2], op=ALU.mult)
            while done_b < T1 // 4:
                phase_b_group(done_b)
                done_b += 1

            if rep is not None:
                rep.__exit__(None, None, None)
            if tiny is not None:
                tz = wp.tile([1, 1], F32, tag="tz")
                nc.vector.memset(tz[:], 0.0)
                nc.sync.dma_start(tiny[:], tz[:])
    nc.finalize()
    return nc


def _host_mats(wv1, bv1, w1, b1, wv2, bv2, w2, b2, wfc, bfc):
    w1m = w1[:, 0, :].astype(np.float32)                    # [32, 3]
    a1 = np.zeros((N + 1, N), np.float32)                   # stage-1 conv
    for j in range(N):
        for dj in range(3):
            jp = j - 1 + dj
            if 0 <= jp < N:
                a1[jp, j] = wv1[dj]
    a1[N, :] = bv1[0]

    mbig = np.zeros((97, 1024), np.float32)
    cidx = np.arange(32) * 32
    for j in range(32):
        for dj in range(3):
            jp = j - 1 + dj
            if 0 <= jp < 32:
                for dk in range(3):
                    mbig[jp * 3 + dk, cidx + j] = wv2[dj] * w1m[:, dk] / 32.0
    for j in range(32):
        s = sum(wv2[dj] for dj in range(3) if 0 <= j - 1 + dj < 32)
        mbig[96, cidx + j] = bv2[0] + b1 * s

    w2k3 = np.zeros((32, 192), np.float32)
    for k in range(3):
        w2k3[:, 64 * k:64 * k + 64] = w2[:, :, k].T / 32.0

    wfcb = np.zeros((65, 512), np.float32)
    wfcb[:64] = wfc.T
    wfcb[64] = bfc + wfc @ b2
    return a1, mbig.astype(NPBF16), w2k3.astype(NPBF16), wfcb.astype(NPBF16)


def kernel(vp, nb1, nb2, wv1, bv1, w1, b1, wv2, bv2, w2, b2, wfc, bfc):
    vp = np.ascontiguousarray(np.asarray(vp, dtype=np.float32))
    nb1 = np.ascontiguousarray(np.asarray(nb1).astype(np.int32))
    nb2 = np.ascontiguousarray(np.asarray(nb2).astype(np.int32))
    wv1 = np.asarray(wv1, np.float32); bv1 = np.asarray(bv1, np.float32)
    w1 = np.asarray(w1, np.float32); b1 = np.asarray(b1, np.float32)
    wv2 = np.asarray(wv2, np.float32); bv2 = np.asarray(bv2, np.float32)
    w2 = np.asarray(w2, np.float32); b2 = np.asarray(b2, np.float32)
    wfc = np.asarray(wfc, np.float32); bfc = np.asarray(bfc, np.float32)

    a1, mbig, w2k3, wfcb = _host_mats(wv1, bv1, w1, b1, wv2, bv2, w2, b2,
                                      wfc, bfc)

    if "s1" not in _CACHE:
        _CACHE["s1"] = _build_stage1()
    if "s2" not in _CACHE:
        _CACHE["s2"] = _build_stage2()

    core_ids = list(range(NCORES))

    # ---- stage 1 ----
    # The first execution of a freshly-loaded NEFF is occasionally corrupted
    # (transient device/runtime flake): warm up once, then validate + retry.
    in1 = [{"vp": vp, "nb1": nb1[VC * c:VC * (c + 1)], "a1": a1}
           for c in range(NCORES)]
    t0 = time.time()
    run_bass_kernel_spmd(_CACHE["s1"], in1, core_ids=core_ids)
    for _ in range(3):
        res1 = run_bass_kernel_spmd(_CACHE["s1"], in1, core_ids=core_ids)
        if all(np.isfinite(res1.results[c]["hsh"]).all()
               for c in range(NCORES)):
            break
    TIMES["stage1_wall"] = time.time() - t0
    hp = np.zeros(V + 2, np.float32)
    for c in range(NCORES):
        hp[1 + VC * c:1 + VC * (c + 1)] = res1.results[c]["hsh"]

    # ---- stage 2 ----
    in2 = []
    for c in range(NCORES):
        vstart = VC * c
        nb2e = np.zeros((EXT, N), np.int32)
        lo = max(vstart - 1, 0)
        hi = min(vstart + VC + 1, V)
        nb2e[lo - (vstart - 1):hi - (vstart - 1)] = nb2[lo:hi]
        mask2 = np.ones((32, 2), np.float32)
        if c == 0:
            mask2[:, 0] = 0.0
        if c == NCORES - 1:
            mask2[:, 1] = 0.0
        in2.append({"hp": hp, "nb2e": nb2e, "mbig": mbig, "w2k3": w2k3,
                    "wfcb": wfcb, "mask2": mask2.astype(NPBF16)})
    global _LAST_INPUTS
    _LAST_INPUTS = (in1, in2)
    t0 = time.time()
    run_bass_kernel_spmd(_CACHE["s2"], in2, core_ids=core_ids)
    for _ in range(3):
        res2 = run_bass_kernel_spmd(_CACHE["s2"], in2, core_ids=core_ids)
        outs = [res2.results[c]["out"] for c in range(NCORES)]
        if all(np.isfinite(o).all() and o.min() >= 0.0 and o.max() <= 1.001
               for o in outs):
            break
    TIMES["stage2_wall"] = time.time() - t0
    return np.concatenate(outs, axis=0)
